# revision 1
# baseline (speedup 1.0000x reference)
"""Trainium2 Bass kernel for the pairwise contact-map decoder.

Reference computation (per batch b):
    tmp[b,i,c,h] = sum_a z[b,i,a] * W1[(a,c),h]
    h1[b,i,j,h]  = relu(sum_c tmp[b,i,c,h] * z[b,j,c] + b1[h])
    h2[b,i,j,k]  = relu(sum_h h1[b,i,j,h] * W2[h,k] + b2[k])
    logit[b,i,j] = (sum_k h2[b,i,j,k] * W3[k,0] + b3) * motif[b,i] * motif[b,j]
    cmap         = sigmoid(logit)

Sparsity: logits are multiplied by motif[i]*motif[j]; rows/cols with
motif == 0 give logit == 0 exactly and cmap == sigmoid(0) == 0.5 exactly.
The host compacts each batch to its nonzero-motif rows (~130-140 of 256
for the thresholded masks this model uses), pads to M=144, runs the
pair-grid MLP on the compacted M x M grid only, and scatters the result
back into a zero/0.5-prefilled full (B, N, N) output.  If a batch ever
has more than 144 nonzero-motif rows, an M=256 variant (the full grid,
identical math) is built instead, so the kernel is exact for arbitrary
mask values.

Sharding: 8 cores, each takes M/2 contiguous compacted i-rows of one
batch (core = 2*b + half). Weights and compacted z[b] are replicated.

On-core dataflow (per core: R = M/2 i-rows, j-width M):
  stage A (fp16 matmuls on 4 concurrent 32-row PE strips via
           tile_position; W1 host-interleaved to 128 partitions for a
           full-bandwidth load): tmp2[i, c, h] = ziT.T @ W1, staged to an
           fp16 DRAM scratch with an extra c-row holding b1 (bias folded
           via K=33).
  per i-pair (fp16 matmul inputs, fp32 PSUM accumulate):
              stage B  h1T[h,(i,j)] = tmp2_i.T @ zTx  (K=33 includes bias);
                       the thin-K matmuls run two-at-a-time on 64-row PE
                       strips (tile_position 0/64, duplicate zTx/tp copies
                       at partitions 0:33 and 64:97), concurrent pairs
                       writing different PSUM banks
              stage C  h2T[k,(i,j)] accumulate over 4 h-chunks of W2
              stage D  logits strip (1, 2M) via W3 chunks
  Emission order per iteration p is B(p), D(p-3), C(p-2) (dC=2): the
  in-order PE then never waits on the DVE relu of h1T or the Act relu of
  h2T, and every cross-engine semaphore wait on the PE stream (measured
  ~0.3-0.6us each on HW) is satisfied long before it is reached.
  Stage D packs 4 pairs' logit strips into ONE PSUM bank at partition
  offsets 0/32/64/96 via col tile_position: 4x fewer PSUM allocations
  and WAR waits.  DMA diet: tp loads batched x2 pairs; logit strips x4
  pairs per DMA; stage-A chunks x4 per DMA -- HWDGE queue slots
  (~625ns) and sequencer DMA-issue (~670ns) dominate small transfers.
  epilogue: mask-mul (outer mask built once on DVE), sigmoid, DMA out,
  in three row-parts as soon as their logits land.
"""

import numpy as np

import concourse.bass as bass
import concourse.mybir as mybir
import concourse.tile as tile
from concourse import bacc
from concourse.bass_utils import run_bass_kernel_spmd

B, N, D, H = 4, 256, 32, 512
DT = mybir.dt
F32, F32R, F16 = DT.float32, DT.float32r, DT.float16
AF = mybir.ActivationFunctionType
ALU = mybir.AluOpType
NCORES = 8
M = 144  # compacted pair-grid width (max nonzero-motif rows + padding)

_cached_nc = {}


from contextlib import nullcontext as _nullcontext


def _r(ap):
    return ap.bitcast(F32R)


def _build(reps=1, m=M, parts="ABCD", dC=3):
    R = m // 2  # i-rows per core
    npair = R // 2
    nc = bacc.Bacc("TRN2", target_bir_lowering=False, debug=False, num_devices=NCORES)

    # all weight layouts/casts are precomputed on the host in _in_maps so
    # every load here is a plain contiguous DMA
    ziT4 = nc.dram_tensor("ziT4", [128, R], F16, kind="ExternalInput")
    zTx = nc.dram_tensor("zTx", [97, m], F16, kind="ExternalInput")
    W1 = nc.dram_tensor("W1", [128, D // 4, H], F16, kind="ExternalInput")
    W2 = nc.dram_tensor("W2", [128, 4, H // 2], F16, kind="ExternalInput")
    W3 = nc.dram_tensor("W3", [128, 2], F16, kind="ExternalInput")
    b1 = nc.dram_tensor("b1", [1, H], F16, kind="ExternalInput")
    b2 = nc.dram_tensor("b2", [128, 2], F32, kind="ExternalInput")
    b3 = nc.dram_tensor("b3", [1], F32, kind="ExternalInput")
    mi = nc.dram_tensor("mi", [R, 1], F32, kind="ExternalInput")
    mj = nc.dram_tensor("mj", [1, m], F32, kind="ExternalInput")
    logits_o = nc.dram_tensor("logits", [R, m], F32, kind="ExternalOutput")
    cmap_o = nc.dram_tensor("cmap", [R, m], F32, kind="ExternalOutput")
    # scratch holding tmp2 transposed per i: (i, c, h) with c=32 rows + b1 row
    tmp2x = nc.dram_tensor("tmp2x", [R, D + 1, H], F16)

    with tile.TileContext(nc) as tc:
        with (
            tc.tile_pool(name="const", bufs=1) as cp,
            tc.tile_pool(name="work", bufs=3) as wp,
            tc.tile_pool(name="ps", bufs=2, space="PSUM") as ps,
        ):
          with tc.For_i(0, reps, 1) if reps > 1 else _nullcontext():
              # ---------- persistent loads ----------
              # bias row of the scratch: tmp2x[:, D, :] = b1 for every i;
              # first on the otherwise-empty gpsimd queue since tp(0)'s bulk
              # read covers it
              nc.gpsimd.dma_start(
                  tmp2x.ap()[:, D, :], b1.ap().broadcast_to([R, H])
              )
              # ziT pre-replicated to all four 32-row strips (host) so stage-A
              # matmuls can use tile_position row-strips; W1 pre-interleaved
              # (partition = 32*(c%4) + a) so the load runs full-width
              ziT4_s = cp.tile([128, R], F16)
              nc.sync.dma_start(ziT4_s[:], ziT4.ap())
              W1_s = cp.tile([128, D // 4, H], F16)
              # cg groups in stage-A consumption order (chunks 0:4 last)
              nc.sync.dma_start(W1_s[:, 1:5, :], W1.ap()[:, 1:5, :])
              nc.scalar.dma_start(W1_s[:, 5:8, :], W1.ap()[:, 5:8, :])
              nc.scalar.dma_start(W1_s[:, 0:1, :], W1.ap()[:, 0:1, :])
              b2_s = cp.tile([128, 2], F32)
              nc.scalar.dma_start(b2_s[:], b2.ap())
              b3_s = cp.tile([1, 1], F32)
              nc.scalar.dma_start(b3_s[:], b3.ap().unsqueeze(0))
              b3_s4 = cp.tile([4, 1], F32)
              nc.scalar.dma_start(b3_s4[:], b3.ap().unsqueeze(0).broadcast_to([4, 1]))
              mi_s = cp.tile([R, 1], F32)
              nc.scalar.dma_start(mi_s[:], mi.ap())
              zTx_s = cp.tile([97, m], F16)
              nc.scalar.dma_start(zTx_s[:], zTx.ap())
              W2_s = cp.tile([128, 4, H // 2], F16)
              nc.scalar.dma_start(W2_s[:], W2.ap())
              W3_s = cp.tile([128, 2], F16)
              nc.scalar.dma_start(W3_s[:], W3.ap())
              mjb = cp.tile([R, m], F32)
              nc.scalar.dma_start(mjb[:], mj.ap().broadcast_to([R, m]))
              logits_sb = cp.tile([R, m], F32)

              # outer motif mask, built once on DVE (no PSUM bank needed)
              mask_sb = cp.tile([R, m], F32)
              nc.vector.tensor_scalar(mask_sb[:], mjb[:], mi_s[:], None, ALU.mult)

              # ---------- stage A: tmp2x[:, c, :] ----------
              # one c-chunk per PSUM tile, 4-deep across the (idle) stage-C/D
              # bank tags so PE never waits on the DVE/Act evictions; four
              # chunks per write-DMA.  Chunks 0:4 are computed LAST so the
              # bulk of tp(0)'s read (c 4:33) can start while they finish and
              # only a small read rides the tail.
              order = [(n, n + 1) for n in list(range(4, D, 2)) + [0, 2]] if "A" in parts else []
              sbA = None
              for k, (n0, n1) in enumerate(order):
                  psA = ps.tile([R, 2, H], F32, tag="b", padded_shape=[128, 2, 512])
                  for t, n in enumerate((n0, n1)):
                      nc.tensor.matmul(
                          psA[:, t, :],
                          ziT4_s[32 * (n % 4) : 32 * (n % 4) + 32, :],
                          W1_s[32 * (n % 4) : 32 * (n % 4) + 32, n // 4, :],
                          start=True,
                          stop=True,
                          tile_position=(32 * (n % 4), 0),
                      )
                  if k % 2 == 0:
                      sbA = wp.tile([R, 4, H], F16, tag="sa")
                      nc.vector.tensor_copy(sbA[:, 0:2, :], psA[:])
                  else:
                      nc.scalar.copy(sbA[:, 2:4, :], psA[:])
                      nc.sync.dma_start(tmp2x.ap()[:, n0 - 2 : n0 + 2, :], sbA[:])

              # ---------- main loop over i-pairs (software-pipelined) ----------
              # tp prefetch: issue each pair-group's load two groups ahead so
              # stage B never waits on the DMA chain
              tp_tiles = {}

              def load_tp(g):
                  tpq = nc.gpsimd if "G" in parts else nc.sync
                  tp = wp.tile([97, 4, H], F16, tag="tp", bufs=4, name="tp")
                  src = tmp2x.ap()[4 * g : 4 * g + 4].rearrange("i c h -> c i h")
                  if g == 0:
                      # c 4:33 can load while stage A finishes chunks 0:4;
                      # the duplicate strip copy rides the idle gpsimd queue
                      tpq.dma_start(tp[4:33], src[4:])
                      nc.gpsimd.dma_start(tp[68:97], src[4:])
                      tpq.dma_start(tp[0:4], src[0:4])
                      nc.gpsimd.dma_start(tp[64:68], src[0:4])
                  else:
                      tpq.dma_start(tp[0:33], src)
                      nc.gpsimd.dma_start(tp[64:97], src)
                  tp_tiles[g] = tp

              # ablation dummies so skipped stages still feed later ones
              if "B" not in parts or "X" in parts:
                  h1T_dummy = cp.tile([128, 4, 2 * m], F16)
                  nc.vector.memset(h1T_dummy[:], 0.25)
              if "D" not in parts:
                  nc.vector.memset(logits_sb[:], 0.0)
              if "C" not in parts:
                  h2T_dummy = cp.tile([128, 2, 2 * m], F16)
                  nc.vector.memset(h2T_dummy[:], 0.25)
              tp_cur = [None]

              def stage_B(p):
                  if "B" in parts and p % 2 == 0:
                      g = p // 2
                      if g not in tp_tiles:
                          load_tp(g)
                      for gn in (g + 1, g + 2):
                          if gn <= (npair - 1) // 2 and gn not in tp_tiles:
                              load_tp(gn)
                      tp_cur[0] = tp_tiles[g]
                  tp = tp_cur[0]
                  if "B" not in parts:
                      return h1T_dummy
                  h1T = None if "Z" in parts else wp.tile([128, 4, 2 * m], F16, tag="h1", bufs=5)
                  for i in range(2):
                      ii = 2 * (p % 2) + i
                      psB = ps.tile([128, 4, m], F32, tag="b", padded_shape=[128, 4, 256])
                      # emission order (0,2,1,3): hc0/hc1 on row-strip 0,
                      # hc2/hc3 on row-strip 64 -- pairs (0,2) and (1,3) run
                      # CONCURRENTLY in the PE array and write different
                      # PSUM banks
                      for hc in (0, 2, 1, 3):
                          sb = 0 if hc < 2 else 64
                          nc.tensor.matmul(
                              psB[:, hc, :],
                              tp[sb : sb + 33, ii, hc * 128 : (hc + 1) * 128],
                              zTx_s[sb : sb + 33, :],
                              start=(hc % 2 == 0),
                              stop=(hc % 2 == 1),
                              tile_position=(sb, 0),
                          )
                      # relu; bias already folded in via the K=33 ones row
                      if "Z" not in parts:
                          nc.vector.tensor_scalar(
                              h1T[:, :, i * m : (i + 1) * m], psB[:], 0.0, None, ALU.max
                          )
                  return h1T_dummy if ("Z" in parts or "X" in parts) else h1T

              def stage_C(p, h1T):
                  if "C" not in parts:
                      return h2T_dummy
                  h2T = wp.tile([128, 2, 2 * m], F16, tag="h2", bufs=5)
                  for kc in range(2):
                      psC = ps.tile([128, 2 * m], F32, tag="ac")
                      for hc in range(4):
                          nc.tensor.matmul(
                              psC[:],
                              W2_s[:, hc, kc * 128 : (kc + 1) * 128],
                              h1T[:, hc, :],
                              start=(hc == 0),
                              stop=(hc == 3),
                          )
                      nc.scalar.activation(
                          h2T[:, kc, :], psC[:], AF.Relu, bias=b2_s[:, kc : kc + 1]
                      )
                  return h2T

              strip4 = [None]
              psD4 = [None]

              def stage_D(p, h2T):
                  if "D" not in parts:
                      return
                  g = p % 4
                  if g == 0:
                      # four pairs' logit strips share one PSUM bank at
                      # partition offsets 0/32/64/96 (col tile_position):
                      # 4x fewer PSUM allocations, WAR waits and evictions
                      psD4[0] = ps.tile([97, 2 * m], F32, tag="d", bufs=2, name="psD4")
                  out = psD4[0][32 * g : 32 * g + 1, :]
                  nc.tensor.matmul(out, W3_s[:, 0:1], h2T[:, 0, :], start=True,
                                   stop=False, tile_position=(0, 32 * g))
                  nc.tensor.matmul(out, W3_s[:, 1:2], h2T[:, 1, :], start=False,
                                   stop=True, tile_position=(0, 32 * g))
                  if g == 0:
                      strip4[0] = wp.tile([97, 2 * m], F32, tag="st", name="strip4")
                  nc.scalar.activation(
                      strip4[0][32 * g : 32 * g + 1, :], out, AF.Identity, bias=b3_s[:]
                  )
                  if g == 3 or p == npair - 1:
                      eng = nc.scalar if p >= npair - 4 else nc.sync
                      eng.dma_start(
                          logits_sb[2 * (p - g) : 2 * p + 2, :],
                          strip4[0][0 : 32 * g + 1 : 32, :],
                      )

              # epilogue runs in row-parts as soon as their logits land;
              # part boundaries must be 32-partition aligned for DVE/Act APs
              mlog = cp.tile([R, m], F32)
              cmap_sb = cp.tile([R, m], F32)
              eparts = [(0, 32, 15), (32, 64, 31), (64, R, npair - 1)]

              def epilogue_part(k):
                  rows = slice(eparts[k][0], eparts[k][1])
                  nc.vector.tensor_mul(mlog[rows, :], logits_sb[rows, :], mask_sb[rows, :])
                  nc.sync.dma_start(logits_o.ap()[rows, :], mlog[rows, :])
                  nc.scalar.activation(cmap_sb[rows, :], mlog[rows, :], AF.Sigmoid)
                  eng = nc.scalar if k == len(eparts) - 1 else nc.sync
                  eng.dma_start(cmap_o.ap()[rows, :], cmap_sb[rows, :])

              # C lags B by dC pairs, D lags by dC+1: the in-order PE never
              # waits on the DVE/Act evictions even with real semaphore
              # latency.  Tile buffer counts (h1/h2 bufs=3) support dC <= 2.
              dD = dC + 1
              h1Ts = {}
              h2Ts = {}

              def emit_D(q):
                  stage_D(q, h2Ts.pop(q))
                  if q == eparts[0][2]:
                      epilogue_part(0)
                  if q == eparts[1][2]:
                      epilogue_part(1)

              for p in range(npair):
                  h1Ts[p] = stage_B(p)
                  if p >= dD:
                      emit_D(p - dD)
                  if p >= dC:
                      h2Ts[p - dC] = stage_C(p - dC, h1Ts.pop(p - dC))
              for p in range(npair, npair + dC):
                  if p >= dD:
                      emit_D(p - dD)
                  h2Ts[p - dC] = stage_C(p - dC, h1Ts.pop(p - dC))
              for q in range(npair + dC - dD, npair):
                  emit_D(q)
              epilogue_part(2)

    nc.compile()
    return nc


def _compact_idx(motif_mask):
    """Per-batch indices of nonzero-motif rows."""
    motif_mask = np.asarray(motif_mask, dtype=np.float32)
    return [np.flatnonzero(motif_mask[b] != 0.0) for b in range(B)]


def _in_maps(z, motif_mask, W1, b1, W2, b2, W3, b3, m=M, idxs=None):
    z = np.ascontiguousarray(np.asarray(z, dtype=np.float32))
    motif_mask = np.asarray(motif_mask, dtype=np.float32)
    if idxs is None:
        idxs = _compact_idx(motif_mask)
    R = m // 2
    # weight layouts/casts precomputed here so device loads are plain DMAs:
    # W1 interleave: partition = 32*(c%4) + a, free = (c//4, h)
    W1 = np.asarray(W1, dtype=np.float32).reshape(D, D // 4, 4, H)
    W1x = np.ascontiguousarray(
        W1.transpose(2, 0, 1, 3).reshape(128, D // 4, H).astype(np.float16)
    )
    W2x = np.ascontiguousarray(
        np.asarray(W2, dtype=np.float32).reshape(4, 128, H // 2).transpose(1, 0, 2)
        .astype(np.float16)
    )
    W3x = np.ascontiguousarray(
        np.asarray(W3, dtype=np.float32).reshape(2, 128).T.astype(np.float16)
    )
    b1x = np.asarray(b1, dtype=np.float32).reshape(1, H).astype(np.float16)
    b2x = np.ascontiguousarray(np.asarray(b2, dtype=np.float32).reshape(2, 128).T)
    b3 = np.ascontiguousarray(np.asarray(b3, dtype=np.float32)).reshape(1)
    maps = []
    for c in range(NCORES):
        b, half = divmod(c, 2)
        idx = idxs[b]
        zc = np.zeros((m, D), np.float32)
        zc[: len(idx)] = z[b, idx]
        mc = np.zeros(m, np.float32)
        mc[: len(idx)] = motif_mask[b, idx]
        rows = slice(half * R, (half + 1) * R)
        zTx1 = np.concatenate([zc.T, np.ones((1, m), np.float32)], axis=0)
        zTx = np.zeros((97, m), np.float32)
        zTx[0:33] = zTx1
        zTx[64:97] = zTx1
        maps.append(
            {
                "ziT4": np.ascontiguousarray(
                    np.tile(zc[rows].T.astype(np.float16), (4, 1))
                ),
                "zTx": np.ascontiguousarray(zTx.astype(np.float16)),
                "W1": W1x,
                "W2": W2x,
                "W3": W3x,
                "b1": b1x,
                "b2": b2x,
                "b3": b3,
                "mi": np.ascontiguousarray(mc[rows].reshape(R, 1)),
                "mj": np.ascontiguousarray(mc.reshape(1, m)),
            }
        )
    return maps


def kernel(z, motif_mask, residue_mask, W1, b1, W2, b2, W3, b3):
    global _cached_nc
    idxs = _compact_idx(motif_mask)
    m = M if max(len(ix) for ix in idxs) <= M else N
    if m not in _cached_nc:
        _cached_nc[m] = _build(m=m)
        if m == M:
            _cached_nc[1] = _cached_nc[m]
    nc = _cached_nc[m]

    maps = _in_maps(z, motif_mask, W1, b1, W2, b2, W3, b3, m=m, idxs=idxs)
    res = run_bass_kernel_spmd(nc, maps, list(range(NCORES)))

    logits = np.zeros((B, N, N), np.float32)
    cmap = np.full((B, N, N), 0.5, np.float32)
    for b in range(B):
        idx = idxs[b]
        cnt = len(idx)
        Lb = np.concatenate(
            [res.results[2 * b]["logits"], res.results[2 * b + 1]["logits"]], axis=0
        )[:cnt, :cnt]
        Cb = np.concatenate(
            [res.results[2 * b]["cmap"], res.results[2 * b + 1]["cmap"]], axis=0
        )[:cnt, :cnt]
        logits[b][np.ix_(idx, idx)] = Lb
        cmap[b][np.ix_(idx, idx)] = Cb
    return cmap, logits

